# revision 25
# baseline (speedup 1.0000x reference)
"""Trainium2 Bass kernel for ClebschCombiningSingleUnrolled (segment_reduce).

out[mu_k] += mult_k * X1[m1_k] * X2[m2_k]   summed over k, per (n, d) element.

Shapes (hardcoded): X1, X2: [9, 4096, 256] f32; index lists: [100]; out: [9, 4096, 256] f32.
Sharding: N (dim 1) split across 8 NeuronCores; index math is host-side.
"""

import os
import sys
import functools

import numpy as np

sys.path.insert(0, "/opt/trn_rl_repo")
os.environ.setdefault("MYCRO_LOCAL_CACHE", "1")

import concourse.bass as bass  # noqa: E402
import concourse.bacc as bacc  # noqa: E402
import concourse.tile as tile  # noqa: E402
from concourse import mybir  # noqa: E402
from concourse.bass_utils import run_bass_kernel_spmd  # noqa: E402

M = 9
N = 4096
D = 256
K = 100
NCORES = 8
NS = N // NCORES  # 512 environment pairs per core
F32 = mybir.dt.float32

MULT = mybir.AluOpType.mult
ADD = mybir.AluOpType.add


def _plan(m1, m2, mu, mult):
    """Group the K terms: merge exact (a,b,mu) duplicates, then group by (a,b) pair.

    Returns list of (a, b, [(mu, w), ...]).
    """
    merged = {}
    for a, b, m, w in zip(m1, m2, mu, mult):
        key = (int(a), int(b), int(m))
        merged[key] = merged.get(key, 0.0) + float(w)
    pairs = {}
    for (a, b, m), w in merged.items():
        pairs.setdefault((a, b), []).append((m, w))
    return [(a, b, uses) for (a, b), uses in sorted(pairs.items())]


def _build_dve_kernel(m1, m2, mu, mult):
    """Phase-1 kernel: layout C (n on partitions, (m, d) on free), all work on DVE.

    Exact fp32. Per 128-row n-chunk: memset acc, one product per unique (a,b)
    pair, one fused scale-and-accumulate (scalar_tensor_tensor) per term.
    """
    plan = _plan(m1, m2, mu, mult)

    nc = bacc.Bacc(trn_type="TRN2")
    x1_d = nc.dram_tensor("X1", [M, NS, D], F32, kind="ExternalInput")
    x2_d = nc.dram_tensor("X2", [M, NS, D], F32, kind="ExternalInput")
    out_d = nc.dram_tensor("OUT", [M, NS, D], F32, kind="ExternalOutput")

    n_chunks = NS // 128

    with tile.TileContext(nc) as tc:
        with (
            tc.tile_pool(name="io", bufs=2) as io_pool,
            tc.tile_pool(name="acc", bufs=2) as acc_pool,
            tc.tile_pool(name="tmp", bufs=2) as tmp_pool,
        ):
            for c in range(n_chunks):
                n0 = c * 128
                x1t = io_pool.tile([128, M, D], F32, tag="x1t")
                nc.gpsimd.dma_start(
                    x1t[:], x1_d[:, n0 : n0 + 128, :].rearrange("m n d -> n m d")
                )
                x2t = io_pool.tile([128, M, D], F32, tag="x2t")
                nc.gpsimd.dma_start(
                    x2t[:], x2_d[:, n0 : n0 + 128, :].rearrange("m n d -> n m d")
                )

                # Absorb each DMA-completion wait into its own 1-element DVE op:
                # the TensorTensor ISA struct can't encode 2 sem waits, so the
                # first real consumer of (x1t, x2t) must not be the one waiting.
                sink = tmp_pool.tile([1, 2], F32, tag="sink", name="sink")
                nc.vector.tensor_copy(sink[:, 0:1], x1t[0:1, 0, 0:1])
                nc.vector.tensor_copy(sink[:, 1:2], x2t[0:1, 0, 0:1])

                acc = [
                    acc_pool.tile([128, D], F32, tag=f"acc{m}", name=f"acc{m}")
                    for m in range(M)
                ]
                written = [False] * M

                for a, b, uses in plan:
                    if len(uses) == 1:
                        m, w = uses[0]
                        if not written[m]:
                            nc.vector.scalar_tensor_tensor(
                                acc[m][:], x1t[:, a, :], float(w), x2t[:, b, :],
                                MULT, MULT,
                            )
                            written[m] = True
                        else:
                            tmp = tmp_pool.tile([128, D], F32)
                            nc.vector.scalar_tensor_tensor(
                                tmp[:], x1t[:, a, :], float(w), x2t[:, b, :],
                                MULT, MULT,
                            )
                            nc.vector.scalar_tensor_tensor(
                                acc[m][:], tmp[:], 1.0, acc[m][:], MULT, ADD
                            )
                    else:
                        tmp = tmp_pool.tile([128, D], F32)
                        nc.vector.tensor_mul(tmp[:], x1t[:, a, :], x2t[:, b, :])
                        for m, w in uses:
                            if not written[m]:
                                nc.vector.tensor_scalar_mul(
                                    acc[m][:], tmp[:], float(w)
                                )
                                written[m] = True
                            else:
                                nc.vector.scalar_tensor_tensor(
                                    acc[m][:], tmp[:], float(w), acc[m][:], MULT, ADD
                                )

                for m in range(M):
                    if not written[m]:
                        nc.vector.memset(acc[m][:], 0.0)
                    nc.gpsimd.dma_start(out_d[m, n0 : n0 + 128, :], acc[m][:])

    nc.compile()
    return nc


BF16 = mybir.dt.bfloat16
SQUARE = mybir.ActivationFunctionType.Square
F1 = 512  # fp32 elements per PSUM bank / per chunk
S = NS * D  # flattened (n, d) extent per core, per m-row


def _sq_matrices(m1, m2, mu, mult):
    """Host-side gather/scatter matrices for the difference-of-squares kernel.

    x*y = ((x+y)^2 - x^2 - y^2) / 2 per unique (a, b) pair, with the x^2 / y^2
    squares shared across pairs.  Rows of the gathered tile SD:
      0..U-1 : X1[a_p] + X2[b_p]     (via G columns with two ones)
      U..U+8 : X1[a]                 (a = 0..8)
      U+9..U+17 : X2[b]              (b = 0..8)
    out[m] = sum_r W[r, m] * SD[r]^2.
    """
    plan = _plan(m1, m2, mu, mult)
    U = len(plan)
    R = U + 18
    G = np.zeros((18, R), np.float32)
    W = np.zeros((R, 9), np.float32)
    for p, (a, b, uses) in enumerate(plan):
        G[a, p] = 1.0
        G[9 + b, p] = 1.0
        for m, w in uses:
            wt = 0.5 * w
            W[p, m] += wt
            W[U + a, m] -= wt
            W[U + 9 + b, m] -= wt
    for a in range(9):
        G[a, U + a] = 1.0
        G[9 + a, U + 9 + a] = 1.0
    return G, W, R


def _build_sq_kernel(m1, m2, mu, mult, n_act=NS):
    """Hybrid kernel.

    n in [0, n_act): difference-of-squares pipeline — PE gathers (bf16, 4
    row-tiles) -> ACT Square over 2 PSUM banks -> PE scatters (col-tiles
    packed into one bank) -> DVE copy -> DMA out.

    n in [n_act, NS): direct layout-C pipeline on DVE — bf16 products +
    fp32 scalar_tensor_tensor accumulation, no PE/ACT involvement.

    Inputs DMA-cast fp32->bf16; all accumulation fp32.
    """
    G_np, W_np, R = _sq_matrices(m1, m2, mu, mult)
    plan = _plan(m1, m2, mu, mult)

    nc = bacc.Bacc(trn_type="TRN2", num_swdge_queues=4)
    x1_d = nc.dram_tensor("X1", [M, NS, D], F32, kind="ExternalInput")
    x2_d = nc.dram_tensor("X2", [M, NS, D], F32, kind="ExternalInput")
    g_d = nc.dram_tensor("G", [18, R], F32, kind="ExternalInput")
    w_d = nc.dram_tensor("W", [R, 9], F32, kind="ExternalInput")
    out_d = nc.dram_tensor("OUT", [M, NS, D], F32, kind="ExternalOutput")

    x1_f = x1_d.rearrange("m n d -> m (n d)")
    x2_f = x2_d.rearrange("m n d -> m (n d)")
    out_f = out_d.rearrange("m n d -> m (n d)")

    NB = 8  # banks-per-group per super-batch; chunk (sb, i, b) = f-range
    SUPER = 4 * NB * F1  # f extent per super-batch
    s_act = n_act * D  # flat f extent handled by the square pipeline
    assert s_act % SUPER == 0 and (NS - n_act) % 128 == 0
    n_super = s_act // SUPER

    n_dve = NS - n_act
    NU = n_dve // 128  # 128-n chunks on the DVE side

    with tile.TileContext(nc) as tc:
        with (
            tc.tile_pool(name="wpool", bufs=1) as wpool,
            tc.tile_pool(name="vpool", bufs=2) as vpool,
            tc.tile_pool(name="qpool", bufs=3) as qpool,
            tc.tile_pool(name="stage", bufs=2) as stage_pool,
            tc.tile_pool(name="dvep", bufs=1) as dvepool,
            tc.tile_pool(name="dtmp", bufs=3) as dtmp_pool,
            tc.tile_pool(name="sdp", bufs=2, space="PSUM") as sd_pool,
            tc.tile_pool(name="outp", bufs=2, space="PSUM") as outp_pool,
        ):
            if n_act:
                gt4 = wpool.tile([128, R], BF16, name="gt4")
                for i in range(4):
                    nc.gpsimd.dma_start(gt4[32 * i : 32 * i + 18, :], g_d[:])
                wt = wpool.tile([R, 9], BF16, name="wt")
                nc.gpsimd.dma_start(wt[:], w_d[:])

            # ---- DVE-side setup: tiles, loads, and the deferred op list ----
            dve_ops = []
            if NU:
                x1t = dvepool.tile([128, NU, M, D], BF16, name="x1t")
                x2t = dvepool.tile([128, NU, M, D], BF16, name="x2t")
                accs = [
                    dvepool.tile([128, NU, D], F32, name=f"dacc{m}", tag=f"dacc{m}")
                    for m in range(M)
                ]
                for u in range(NU):
                    n0 = n_act + u * 128
                    nc.gpsimd.dma_start(
                        x1t[:, u, :, :],
                        x1_d[:, n0 : n0 + 128, :].rearrange("m n d -> n m d"),
                    )
                    nc.gpsimd.dma_start(
                        x2t[:, u, :, :],
                        x2_d[:, n0 : n0 + 128, :].rearrange("m n d -> n m d"),
                    )

                written = [False] * M

                def _stt(out_ap, in0_ap, w, in1_ap, op1):
                    return lambda: nc.vector.scalar_tensor_tensor(
                        out_ap, in0_ap, w, in1_ap, MULT, op1
                    )

                for a, b, uses in plan:
                    x1a = x1t[:, :, a, :]
                    x2b = x2t[:, :, b, :]
                    if len(uses) == 1 and not written[uses[0][0]]:
                        m, w = uses[0]
                        dve_ops.append(_stt(accs[m][:], x1a, float(w), x2b, MULT))
                        written[m] = True
                    else:
                        # product into tmp, then one fused op per use
                        def _mk(x1a=x1a, x2b=x2b, uses=tuple(uses)):
                            ops = []
                            state = {}
                            def prod():
                                state["tmp"] = dtmp_pool.tile(
                                    [128, NU, D], BF16, name="dtmp"
                                )
                                nc.vector.tensor_mul(state["tmp"][:], x1a, x2b)
                            ops.append(prod)
                            for m, w in uses:
                                if not written[m]:
                                    def fw(m=m, w=w):
                                        nc.vector.tensor_scalar_mul(
                                            accs[m][:], state["tmp"][:], float(w)
                                        )
                                    ops.append(fw)
                                    written[m] = True
                                else:
                                    def ac(m=m, w=w):
                                        nc.vector.scalar_tensor_tensor(
                                            accs[m][:], state["tmp"][:], float(w),
                                            accs[m][:], MULT, ADD,
                                        )
                                    ops.append(ac)
                            return ops
                        dve_ops.extend(_mk())

                for m in range(M):
                    if not written[m]:
                        mm = m
                        dve_ops.insert(0, lambda mm=mm: nc.vector.memset(accs[mm][:], 0.0))

            def _emit_dve(k0, k1):
                for op in dve_ops[k0:k1]:
                    op()

            # Chunk (i, b) of super-batch sb covers f-range
            #   [f0 + (i*NB + b)*F1, +F1)  with f0 = sb*SUPER —
            # row-group i owns a CONTIGUOUS [9, NB*F1] HBM range (16KB/row).
            ndve_done = 0
            for sb in range(n_super):
                f0 = sb * SUPER
                v4 = vpool.tile([128, NB * F1], BF16, name="v4")
                for i in range(4):
                    fi = f0 + i * NB * F1
                    nc.gpsimd.dma_start(
                        v4[32 * i : 32 * i + 9, :], x1_f[:, fi : fi + NB * F1]
                    )
                    nc.gpsimd.dma_start(
                        v4[32 * i + 9 : 32 * i + 18, :], x2_f[:, fi : fi + NB * F1]
                    )

                stage = stage_pool.tile([128, NB * F1], F32, name="stage")

                # Process in half-batches of 2 chunks (2 PSUM banks) so the
                # SD pool double-buffers within the 8-bank budget:
                # groups (0,1) then (2,3) for each b.
                for b in range(NB):
                    outp = outp_pool.tile([128, F1], F32, name="outp", tag="outp")
                    for h in range(2):
                        sd = sd_pool.tile([R, 2, F1], F32, name="sd")
                        for j in range(2):
                            i = 2 * h + j
                            nc.tensor.matmul(
                                sd[:, j, :],
                                gt4[32 * i : 32 * i + 18, :],
                                v4[32 * i : 32 * i + 18, b * F1 : (b + 1) * F1],
                                start=True,
                                stop=True,
                                tile_position=(32 * i, 0),
                            )
                        qs = qpool.tile([R, 2, F1], BF16, name="qs")
                        nc.scalar.activation(
                            qs[:].rearrange("p a t -> p (a t)"),
                            sd[:].rearrange("p a t -> p (a t)"),
                            SQUARE,
                        )
                        for j in range(2):
                            i = 2 * h + j
                            nc.tensor.matmul(
                                outp[32 * i : 32 * i + 9, :],
                                wt[:],
                                qs[:, j, :],
                                start=True,
                                stop=True,
                                tile_position=(0, 32 * i),
                            )
                        if h == 1:
                            if os.environ.get("BASS_COPY_ENGINE", "act") == "act":
                                nc.scalar.copy(
                                    stage[:, b * F1 : (b + 1) * F1], outp[:]
                                )
                            else:
                                nc.vector.tensor_copy(
                                    stage[:, b * F1 : (b + 1) * F1], outp[:]
                                )

                for i in range(4):
                    fi = f0 + i * NB * F1
                    nc.sync.dma_start(
                        out_f[:, fi : fi + NB * F1], stage[32 * i : 32 * i + 9, :]
                    )

                k1 = len(dve_ops) * (sb + 1) // max(n_super, 1)
                _emit_dve(ndve_done, k1)
                ndve_done = k1

            _emit_dve(ndve_done, len(dve_ops))
            if NU:
                for u in range(NU):
                    n0 = n_act + u * 128
                    for m in range(M):
                        nc.sync.dma_start(
                            out_d[m, n0 : n0 + 128, :], accs[m][:, u, :]
                        )

    nc.compile()
    return nc


_CACHE = {}


def _get_nc(key, builder, *args):
    if key not in _CACHE:
        _CACHE[key] = builder(*args)
    return _CACHE[key]


def prepare(X1, X2, m1_aligned, m2_aligned, mu, multipliers):
    """Build (or fetch cached) the Bass program and per-core input maps."""
    X1 = np.ascontiguousarray(X1, dtype=np.float32)
    X2 = np.ascontiguousarray(X2, dtype=np.float32)
    m1 = [int(v) for v in np.asarray(m1_aligned)]
    m2 = [int(v) for v in np.asarray(m2_aligned)]
    mus = [int(v) for v in np.asarray(mu)]
    mult = [float(v) for v in np.asarray(multipliers, dtype=np.float32)]

    impl = os.environ.get("BASS_KERNEL_IMPL", "sq")
    n_act = int(os.environ.get("BASS_NA", "256"))
    key = (impl, n_act, tuple(m1), tuple(m2), tuple(mus), tuple(mult))
    if impl == "dve1":
        nc = _get_nc(key, _build_dve_kernel, m1, m2, mus, mult)
        extra = {}
    else:
        nc = _get_nc(key, _build_sq_kernel, m1, m2, mus, mult, n_act)
        G_np, W_np, _ = _sq_matrices(m1, m2, mus, mult)
        extra = {"G": G_np, "W": W_np}

    in_maps = []
    for c in range(NCORES):
        sl = slice(c * NS, (c + 1) * NS)
        in_maps.append(
            {
                "X1": np.ascontiguousarray(X1[:, sl, :]),
                "X2": np.ascontiguousarray(X2[:, sl, :]),
                **extra,
            }
        )
    return nc, in_maps


def kernel(X1, X2, m1_aligned, m2_aligned, mu, multipliers):
    nc, in_maps = prepare(X1, X2, m1_aligned, m2_aligned, mu, multipliers)
    res = run_bass_kernel_spmd(nc, in_maps, core_ids=list(range(NCORES)))
    out = np.concatenate([res.results[c]["OUT"] for c in range(NCORES)], axis=1)
    return out


# revision 29
# speedup vs baseline: 1.1648x; 1.1648x over previous
"""Trainium2 Bass kernel for ClebschCombiningSingleUnrolled (segment_reduce).

out[mu_k] += mult_k * X1[m1_k] * X2[m2_k]   summed over k, per (n, d) element.

Shapes (hardcoded): X1, X2: [9, 4096, 256] f32; index lists: [100]; out: [9, 4096, 256] f32.
Sharding: N (dim 1) split across 8 NeuronCores; index math is host-side.
"""

import os
import sys
import functools

import numpy as np

sys.path.insert(0, "/opt/trn_rl_repo")
os.environ.setdefault("MYCRO_LOCAL_CACHE", "1")

import concourse.bass as bass  # noqa: E402
import concourse.bacc as bacc  # noqa: E402
import concourse.tile as tile  # noqa: E402
from concourse import mybir  # noqa: E402
from concourse.bass_utils import run_bass_kernel_spmd  # noqa: E402

M = 9
N = 4096
D = 256
K = 100
NCORES = 8
NS = N // NCORES  # 512 environment pairs per core
F32 = mybir.dt.float32

MULT = mybir.AluOpType.mult
ADD = mybir.AluOpType.add


def _plan(m1, m2, mu, mult):
    """Group the K terms: merge exact (a,b,mu) duplicates, then group by (a,b) pair.

    Returns list of (a, b, [(mu, w), ...]).
    """
    merged = {}
    for a, b, m, w in zip(m1, m2, mu, mult):
        key = (int(a), int(b), int(m))
        merged[key] = merged.get(key, 0.0) + float(w)
    pairs = {}
    for (a, b, m), w in merged.items():
        pairs.setdefault((a, b), []).append((m, w))
    return [(a, b, uses) for (a, b), uses in sorted(pairs.items())]


def _build_dve_kernel(m1, m2, mu, mult):
    """Phase-1 kernel: layout C (n on partitions, (m, d) on free), all work on DVE.

    Exact fp32. Per 128-row n-chunk: memset acc, one product per unique (a,b)
    pair, one fused scale-and-accumulate (scalar_tensor_tensor) per term.
    """
    plan = _plan(m1, m2, mu, mult)

    nc = bacc.Bacc(trn_type="TRN2")
    x1_d = nc.dram_tensor("X1", [M, NS, D], F32, kind="ExternalInput")
    x2_d = nc.dram_tensor("X2", [M, NS, D], F32, kind="ExternalInput")
    out_d = nc.dram_tensor("OUT", [M, NS, D], F32, kind="ExternalOutput")

    n_chunks = NS // 128

    with tile.TileContext(nc) as tc:
        with (
            tc.tile_pool(name="io", bufs=2) as io_pool,
            tc.tile_pool(name="acc", bufs=2) as acc_pool,
            tc.tile_pool(name="tmp", bufs=2) as tmp_pool,
        ):
            for c in range(n_chunks):
                n0 = c * 128
                x1t = io_pool.tile([128, M, D], F32, tag="x1t")
                nc.gpsimd.dma_start(
                    x1t[:], x1_d[:, n0 : n0 + 128, :].rearrange("m n d -> n m d")
                )
                x2t = io_pool.tile([128, M, D], F32, tag="x2t")
                nc.gpsimd.dma_start(
                    x2t[:], x2_d[:, n0 : n0 + 128, :].rearrange("m n d -> n m d")
                )

                # Absorb each DMA-completion wait into its own 1-element DVE op:
                # the TensorTensor ISA struct can't encode 2 sem waits, so the
                # first real consumer of (x1t, x2t) must not be the one waiting.
                sink = tmp_pool.tile([1, 2], F32, tag="sink", name="sink")
                nc.vector.tensor_copy(sink[:, 0:1], x1t[0:1, 0, 0:1])
                nc.vector.tensor_copy(sink[:, 1:2], x2t[0:1, 0, 0:1])

                acc = [
                    acc_pool.tile([128, D], F32, tag=f"acc{m}", name=f"acc{m}")
                    for m in range(M)
                ]
                written = [False] * M

                for a, b, uses in plan:
                    if len(uses) == 1:
                        m, w = uses[0]
                        if not written[m]:
                            nc.vector.scalar_tensor_tensor(
                                acc[m][:], x1t[:, a, :], float(w), x2t[:, b, :],
                                MULT, MULT,
                            )
                            written[m] = True
                        else:
                            tmp = tmp_pool.tile([128, D], F32)
                            nc.vector.scalar_tensor_tensor(
                                tmp[:], x1t[:, a, :], float(w), x2t[:, b, :],
                                MULT, MULT,
                            )
                            nc.vector.scalar_tensor_tensor(
                                acc[m][:], tmp[:], 1.0, acc[m][:], MULT, ADD
                            )
                    else:
                        tmp = tmp_pool.tile([128, D], F32)
                        nc.vector.tensor_mul(tmp[:], x1t[:, a, :], x2t[:, b, :])
                        for m, w in uses:
                            if not written[m]:
                                nc.vector.tensor_scalar_mul(
                                    acc[m][:], tmp[:], float(w)
                                )
                                written[m] = True
                            else:
                                nc.vector.scalar_tensor_tensor(
                                    acc[m][:], tmp[:], float(w), acc[m][:], MULT, ADD
                                )

                for m in range(M):
                    if not written[m]:
                        nc.vector.memset(acc[m][:], 0.0)
                    nc.gpsimd.dma_start(out_d[m, n0 : n0 + 128, :], acc[m][:])

    nc.compile()
    return nc


BF16 = mybir.dt.bfloat16
SQUARE = mybir.ActivationFunctionType.Square
F1 = 512  # fp32 elements per PSUM bank / per chunk
S = NS * D  # flattened (n, d) extent per core, per m-row


def _sq_matrices(m1, m2, mu, mult):
    """Host-side gather/scatter matrices for the difference-of-squares kernel.

    x*y = ((x+y)^2 - x^2 - y^2) / 2 per unique (a, b) pair, with the x^2 / y^2
    squares shared across pairs.  Rows of the gathered tile SD:
      0..U-1 : X1[a_p] + X2[b_p]     (via G columns with two ones)
      U..U+8 : X1[a]                 (a = 0..8)
      U+9..U+17 : X2[b]              (b = 0..8)
    out[m] = sum_r W[r, m] * SD[r]^2.
    """
    plan = _plan(m1, m2, mu, mult)
    U = len(plan)
    R = U + 18
    G = np.zeros((18, R), np.float32)
    W = np.zeros((R, 9), np.float32)
    for p, (a, b, uses) in enumerate(plan):
        G[a, p] = 1.0
        G[9 + b, p] = 1.0
        for m, w in uses:
            wt = 0.5 * w
            W[p, m] += wt
            W[U + a, m] -= wt
            W[U + 9 + b, m] -= wt
    for a in range(9):
        G[a, U + a] = 1.0
        G[9 + a, U + 9 + a] = 1.0
    return G, W, R


def _build_sq_kernel(m1, m2, mu, mult, n_act=NS):
    """Hybrid kernel.

    n in [0, n_act): difference-of-squares pipeline — PE gathers (bf16, 4
    row-tiles) -> ACT Square over 2 PSUM banks -> PE scatters (col-tiles
    packed into one bank) -> DVE copy -> DMA out.

    n in [n_act, NS): direct layout-C pipeline on DVE — bf16 products +
    fp32 scalar_tensor_tensor accumulation, no PE/ACT involvement.

    Inputs DMA-cast fp32->bf16; all accumulation fp32.
    """
    G_np, W_np, R = _sq_matrices(m1, m2, mu, mult)
    plan = _plan(m1, m2, mu, mult)

    nc = bacc.Bacc(trn_type="TRN2", num_swdge_queues=4)
    x1_d = nc.dram_tensor("X1", [M, NS, D], F32, kind="ExternalInput")
    x2_d = nc.dram_tensor("X2", [M, NS, D], F32, kind="ExternalInput")
    g_d = nc.dram_tensor("G", [18, R], F32, kind="ExternalInput")
    w_d = nc.dram_tensor("W", [R, 9], F32, kind="ExternalInput")
    out_d = nc.dram_tensor("OUT", [M, NS, D], F32, kind="ExternalOutput")

    x1_f = x1_d.rearrange("m n d -> m (n d)")
    x2_f = x2_d.rearrange("m n d -> m (n d)")
    out_f = out_d.rearrange("m n d -> m (n d)")

    NB = 8  # banks-per-group per super-batch; chunk (sb, i, b) = f-range
    SUPER = 4 * NB * F1  # f extent per super-batch
    s_act = n_act * D  # flat f extent handled by the square pipeline
    assert s_act % SUPER == 0 and (NS - n_act) % 128 == 0
    n_super = s_act // SUPER

    n_dve = NS - n_act
    NU = n_dve // 128  # 128-n chunks on the DVE side

    with tile.TileContext(nc) as tc:
        with (
            tc.tile_pool(name="wpool", bufs=1) as wpool,
            tc.tile_pool(name="vpool", bufs=2) as vpool,
            tc.tile_pool(name="qpool", bufs=3) as qpool,
            tc.tile_pool(name="stage", bufs=2) as stage_pool,
            tc.tile_pool(name="dvep", bufs=1) as dvepool,
            tc.tile_pool(name="dtmp", bufs=3) as dtmp_pool,
            tc.tile_pool(name="sdp", bufs=2, space="PSUM") as sd_pool,
            tc.tile_pool(name="outp", bufs=2, space="PSUM") as outp_pool,
        ):
            if n_act:
                gt4 = wpool.tile([128, R], BF16, name="gt4")
                for i in range(4):
                    nc.gpsimd.dma_start(gt4[32 * i : 32 * i + 18, :], g_d[:])
                wt = wpool.tile([R, 9], BF16, name="wt")
                nc.gpsimd.dma_start(wt[:], w_d[:])

            # ---- DVE-side setup: tiles, loads, and the deferred op list ----
            dve_ops = []
            if NU:
                x1t = dvepool.tile([128, NU, M, D], BF16, name="x1t")
                x2t = dvepool.tile([128, NU, M, D], BF16, name="x2t")
                accs = [
                    dvepool.tile([128, NU, D], F32, name=f"dacc{m}", tag=f"dacc{m}")
                    for m in range(M)
                ]

                def _dve_load(u):
                    n0 = n_act + u * 128
                    nc.gpsimd.dma_start(
                        x1t[:, u, :, :],
                        x1_d[:, n0 : n0 + 128, :].rearrange("m n d -> n m d"),
                    )
                    nc.gpsimd.dma_start(
                        x2t[:, u, :, :],
                        x2_d[:, n0 : n0 + 128, :].rearrange("m n d -> n m d"),
                    )

                written = [False] * M

                def _stt(out_ap, in0_ap, w, in1_ap, op1):
                    return lambda: nc.vector.scalar_tensor_tensor(
                        out_ap, in0_ap, w, in1_ap, MULT, op1
                    )

                for a, b, uses in plan:
                    x1a = x1t[:, :, a, :]
                    x2b = x2t[:, :, b, :]
                    if len(uses) == 1 and not written[uses[0][0]]:
                        m, w = uses[0]
                        dve_ops.append(_stt(accs[m][:], x1a, float(w), x2b, MULT))
                        written[m] = True
                    else:
                        # product into tmp, then one fused op per use
                        def _mk(x1a=x1a, x2b=x2b, uses=tuple(uses)):
                            ops = []
                            state = {}
                            def prod():
                                state["tmp"] = dtmp_pool.tile(
                                    [128, NU, D], BF16, name="dtmp"
                                )
                                nc.vector.tensor_mul(state["tmp"][:], x1a, x2b)
                            ops.append(prod)
                            for m, w in uses:
                                if not written[m]:
                                    def fw(m=m, w=w):
                                        nc.vector.tensor_scalar_mul(
                                            accs[m][:], state["tmp"][:], float(w)
                                        )
                                    ops.append(fw)
                                    written[m] = True
                                else:
                                    def ac(m=m, w=w):
                                        nc.vector.scalar_tensor_tensor(
                                            accs[m][:], state["tmp"][:], float(w),
                                            accs[m][:], MULT, ADD,
                                        )
                                    ops.append(ac)
                            return ops
                        dve_ops.extend(_mk())

                for m in range(M):
                    if not written[m]:
                        mm = m
                        dve_ops.insert(0, lambda mm=mm: nc.vector.memset(accs[mm][:], 0.0))

            def _emit_dve(k0, k1):
                for op in dve_ops[k0:k1]:
                    op()

            # Chunk (i, b) of super-batch sb covers f-range
            #   [f0 + (i*NB + b)*F1, +F1)  with f0 = sb*SUPER —
            # row-group i owns a CONTIGUOUS [9, NB*F1] HBM range (16KB/row).
            ndve_done = 0
            for sb in range(n_super):
                f0 = sb * SUPER
                v4 = vpool.tile([128, NB * F1], BF16, name="v4")
                for i in range(4):
                    fi = f0 + i * NB * F1
                    nc.gpsimd.dma_start(
                        v4[32 * i : 32 * i + 9, :], x1_f[:, fi : fi + NB * F1]
                    )
                    nc.gpsimd.dma_start(
                        v4[32 * i + 9 : 32 * i + 18, :], x2_f[:, fi : fi + NB * F1]
                    )
                if sb == 0 and NU:
                    # DVE-side loads AFTER the first square-batch loads so the
                    # sq pipeline's first tiles aren't stuck behind 4.8MB of
                    # DVE-side DMA in the SWDGE queues.
                    for u in range(NU):
                        _dve_load(u)

                stage = stage_pool.tile([128, NB * F1], F32, name="stage")

                # Process in half-batches of 2 chunks (2 PSUM banks) so the
                # SD pool double-buffers within the 8-bank budget:
                # groups (0,1) then (2,3) for each b.
                for b in range(NB):
                    outp = outp_pool.tile([128, F1], F32, name="outp", tag="outp")
                    for h in range(2):
                        sd = sd_pool.tile([R, 2, F1], F32, name="sd")
                        for j in range(2):
                            i = 2 * h + j
                            nc.tensor.matmul(
                                sd[:, j, :],
                                gt4[32 * i : 32 * i + 18, :],
                                v4[32 * i : 32 * i + 18, b * F1 : (b + 1) * F1],
                                start=True,
                                stop=True,
                                tile_position=(32 * i, 0),
                            )
                        qs = qpool.tile([R, 2, F1], BF16, name="qs")
                        nc.scalar.activation(
                            qs[:].rearrange("p a t -> p (a t)"),
                            sd[:].rearrange("p a t -> p (a t)"),
                            SQUARE,
                        )
                        for j in range(2):
                            i = 2 * h + j
                            nc.tensor.matmul(
                                outp[32 * i : 32 * i + 9, :],
                                wt[:],
                                qs[:, j, :],
                                start=True,
                                stop=True,
                                tile_position=(0, 32 * i),
                            )
                        if h == 1:
                            # Alternate the PSUM->SBUF copy between ACT and DVE
                            # so neither engine eats the whole 23us copy cost.
                            if b % 2 == 0:
                                nc.scalar.copy(
                                    stage[:, b * F1 : (b + 1) * F1], outp[:]
                                )
                            else:
                                nc.vector.tensor_copy(
                                    stage[:, b * F1 : (b + 1) * F1], outp[:]
                                )

                for i in range(4):
                    fi = f0 + i * NB * F1
                    nc.sync.dma_start(
                        out_f[:, fi : fi + NB * F1], stage[32 * i : 32 * i + 9, :]
                    )

                k1 = len(dve_ops) * (sb + 1) // max(n_super, 1)
                _emit_dve(ndve_done, k1)
                ndve_done = k1

            if n_super == 0 and NU:
                for u in range(NU):
                    _dve_load(u)
            _emit_dve(ndve_done, len(dve_ops))
            if NU:
                for u in range(NU):
                    n0 = n_act + u * 128
                    for m in range(M):
                        nc.sync.dma_start(
                            out_d[m, n0 : n0 + 128, :], accs[m][:, u, :]
                        )

    nc.compile()
    return nc


_CACHE = {}


def _get_nc(key, builder, *args):
    if key not in _CACHE:
        _CACHE[key] = builder(*args)
    return _CACHE[key]


def prepare(X1, X2, m1_aligned, m2_aligned, mu, multipliers):
    """Build (or fetch cached) the Bass program and per-core input maps."""
    X1 = np.ascontiguousarray(X1, dtype=np.float32)
    X2 = np.ascontiguousarray(X2, dtype=np.float32)
    m1 = [int(v) for v in np.asarray(m1_aligned)]
    m2 = [int(v) for v in np.asarray(m2_aligned)]
    mus = [int(v) for v in np.asarray(mu)]
    mult = [float(v) for v in np.asarray(multipliers, dtype=np.float32)]

    impl = os.environ.get("BASS_KERNEL_IMPL", "sq")
    n_act = int(os.environ.get("BASS_NA", "256"))
    key = (impl, n_act, tuple(m1), tuple(m2), tuple(mus), tuple(mult))
    if impl == "dve1":
        nc = _get_nc(key, _build_dve_kernel, m1, m2, mus, mult)
        extra = {}
    else:
        nc = _get_nc(key, _build_sq_kernel, m1, m2, mus, mult, n_act)
        G_np, W_np, _ = _sq_matrices(m1, m2, mus, mult)
        extra = {"G": G_np, "W": W_np}

    in_maps = []
    for c in range(NCORES):
        sl = slice(c * NS, (c + 1) * NS)
        in_maps.append(
            {
                "X1": np.ascontiguousarray(X1[:, sl, :]),
                "X2": np.ascontiguousarray(X2[:, sl, :]),
                **extra,
            }
        )
    return nc, in_maps


def kernel(X1, X2, m1_aligned, m2_aligned, mu, multipliers):
    nc, in_maps = prepare(X1, X2, m1_aligned, m2_aligned, mu, multipliers)
    res = run_bass_kernel_spmd(nc, in_maps, core_ids=list(range(NCORES)))
    out = np.concatenate([res.results[c]["OUT"] for c in range(NCORES)], axis=1)
    return out


# revision 30
# speedup vs baseline: 1.3670x; 1.1736x over previous
"""Trainium2 Bass kernel for ClebschCombiningSingleUnrolled (segment_reduce).

out[mu_k] += mult_k * X1[m1_k] * X2[m2_k]   summed over k, per (n, d) element.

Shapes (hardcoded): X1, X2: [9, 4096, 256] f32; index lists: [100]; out: [9, 4096, 256] f32.
Sharding: N (dim 1) split across 8 NeuronCores; index math is host-side.
"""

import os
import sys
import functools

import numpy as np

sys.path.insert(0, "/opt/trn_rl_repo")
os.environ.setdefault("MYCRO_LOCAL_CACHE", "1")

import concourse.bass as bass  # noqa: E402
import concourse.bacc as bacc  # noqa: E402
import concourse.tile as tile  # noqa: E402
from concourse import mybir  # noqa: E402
from concourse.bass_utils import run_bass_kernel_spmd  # noqa: E402

M = 9
N = 4096
D = 256
K = 100
NCORES = 8
NS = N // NCORES  # 512 environment pairs per core
F32 = mybir.dt.float32

MULT = mybir.AluOpType.mult
ADD = mybir.AluOpType.add


def _plan(m1, m2, mu, mult):
    """Group the K terms: merge exact (a,b,mu) duplicates, then group by (a,b) pair.

    Returns list of (a, b, [(mu, w), ...]).
    """
    merged = {}
    for a, b, m, w in zip(m1, m2, mu, mult):
        key = (int(a), int(b), int(m))
        merged[key] = merged.get(key, 0.0) + float(w)
    pairs = {}
    for (a, b, m), w in merged.items():
        pairs.setdefault((a, b), []).append((m, w))
    return [(a, b, uses) for (a, b), uses in sorted(pairs.items())]


def _build_dve_kernel(m1, m2, mu, mult):
    """Phase-1 kernel: layout C (n on partitions, (m, d) on free), all work on DVE.

    Exact fp32. Per 128-row n-chunk: memset acc, one product per unique (a,b)
    pair, one fused scale-and-accumulate (scalar_tensor_tensor) per term.
    """
    plan = _plan(m1, m2, mu, mult)

    nc = bacc.Bacc(trn_type="TRN2")
    x1_d = nc.dram_tensor("X1", [M, NS, D], F32, kind="ExternalInput")
    x2_d = nc.dram_tensor("X2", [M, NS, D], F32, kind="ExternalInput")
    out_d = nc.dram_tensor("OUT", [M, NS, D], F32, kind="ExternalOutput")

    n_chunks = NS // 128

    with tile.TileContext(nc) as tc:
        with (
            tc.tile_pool(name="io", bufs=2) as io_pool,
            tc.tile_pool(name="acc", bufs=2) as acc_pool,
            tc.tile_pool(name="tmp", bufs=2) as tmp_pool,
        ):
            for c in range(n_chunks):
                n0 = c * 128
                x1t = io_pool.tile([128, M, D], F32, tag="x1t")
                nc.gpsimd.dma_start(
                    x1t[:], x1_d[:, n0 : n0 + 128, :].rearrange("m n d -> n m d")
                )
                x2t = io_pool.tile([128, M, D], F32, tag="x2t")
                nc.gpsimd.dma_start(
                    x2t[:], x2_d[:, n0 : n0 + 128, :].rearrange("m n d -> n m d")
                )

                # Absorb each DMA-completion wait into its own 1-element DVE op:
                # the TensorTensor ISA struct can't encode 2 sem waits, so the
                # first real consumer of (x1t, x2t) must not be the one waiting.
                sink = tmp_pool.tile([1, 2], F32, tag="sink", name="sink")
                nc.vector.tensor_copy(sink[:, 0:1], x1t[0:1, 0, 0:1])
                nc.vector.tensor_copy(sink[:, 1:2], x2t[0:1, 0, 0:1])

                acc = [
                    acc_pool.tile([128, D], F32, tag=f"acc{m}", name=f"acc{m}")
                    for m in range(M)
                ]
                written = [False] * M

                for a, b, uses in plan:
                    if len(uses) == 1:
                        m, w = uses[0]
                        if not written[m]:
                            nc.vector.scalar_tensor_tensor(
                                acc[m][:], x1t[:, a, :], float(w), x2t[:, b, :],
                                MULT, MULT,
                            )
                            written[m] = True
                        else:
                            tmp = tmp_pool.tile([128, D], F32)
                            nc.vector.scalar_tensor_tensor(
                                tmp[:], x1t[:, a, :], float(w), x2t[:, b, :],
                                MULT, MULT,
                            )
                            nc.vector.scalar_tensor_tensor(
                                acc[m][:], tmp[:], 1.0, acc[m][:], MULT, ADD
                            )
                    else:
                        tmp = tmp_pool.tile([128, D], F32)
                        nc.vector.tensor_mul(tmp[:], x1t[:, a, :], x2t[:, b, :])
                        for m, w in uses:
                            if not written[m]:
                                nc.vector.tensor_scalar_mul(
                                    acc[m][:], tmp[:], float(w)
                                )
                                written[m] = True
                            else:
                                nc.vector.scalar_tensor_tensor(
                                    acc[m][:], tmp[:], float(w), acc[m][:], MULT, ADD
                                )

                for m in range(M):
                    if not written[m]:
                        nc.vector.memset(acc[m][:], 0.0)
                    nc.gpsimd.dma_start(out_d[m, n0 : n0 + 128, :], acc[m][:])

    nc.compile()
    return nc


BF16 = mybir.dt.bfloat16
SQUARE = mybir.ActivationFunctionType.Square
F1 = 512  # fp32 elements per PSUM bank / per chunk
S = NS * D  # flattened (n, d) extent per core, per m-row


def _sq_matrices(m1, m2, mu, mult):
    """Host-side gather/scatter matrices for the difference-of-squares kernel.

    x*y = ((x+y)^2 - x^2 - y^2) / 2 per unique (a, b) pair, with the x^2 / y^2
    squares shared across pairs.  Rows of the gathered tile SD:
      0..U-1 : X1[a_p] + X2[b_p]     (via G columns with two ones)
      U..U+8 : X1[a]                 (a = 0..8)
      U+9..U+17 : X2[b]              (b = 0..8)
    out[m] = sum_r W[r, m] * SD[r]^2.
    """
    plan = _plan(m1, m2, mu, mult)
    U = len(plan)
    R = U + 18
    G = np.zeros((18, R), np.float32)
    W = np.zeros((R, 9), np.float32)
    for p, (a, b, uses) in enumerate(plan):
        G[a, p] = 1.0
        G[9 + b, p] = 1.0
        for m, w in uses:
            wt = 0.5 * w
            W[p, m] += wt
            W[U + a, m] -= wt
            W[U + 9 + b, m] -= wt
    for a in range(9):
        G[a, U + a] = 1.0
        G[9 + a, U + 9 + a] = 1.0
    return G, W, R


def _build_sq_kernel(m1, m2, mu, mult, n_act=NS):
    """Hybrid kernel.

    n in [0, n_act): difference-of-squares pipeline — PE gathers (bf16, 4
    row-tiles) -> ACT Square over 2 PSUM banks -> PE scatters (col-tiles
    packed into one bank) -> DVE copy -> DMA out.

    n in [n_act, NS): direct layout-C pipeline on DVE — bf16 products +
    fp32 scalar_tensor_tensor accumulation, no PE/ACT involvement.

    Inputs DMA-cast fp32->bf16; all accumulation fp32.
    """
    G_np, W_np, R = _sq_matrices(m1, m2, mu, mult)
    plan = _plan(m1, m2, mu, mult)

    nc = bacc.Bacc(trn_type="TRN2", num_swdge_queues=4)
    x1_d = nc.dram_tensor("X1", [M, NS, D], F32, kind="ExternalInput")
    x2_d = nc.dram_tensor("X2", [M, NS, D], F32, kind="ExternalInput")
    g_d = nc.dram_tensor("G", [18, R], F32, kind="ExternalInput")
    w_d = nc.dram_tensor("W", [R, 9], F32, kind="ExternalInput")
    out_d = nc.dram_tensor("OUT", [M, NS, D], F32, kind="ExternalOutput")

    x1_f = x1_d.rearrange("m n d -> m (n d)")
    x2_f = x2_d.rearrange("m n d -> m (n d)")
    out_f = out_d.rearrange("m n d -> m (n d)")

    NB = 8  # banks-per-group per super-batch; chunk (sb, i, b) = f-range
    SUPER = 4 * NB * F1  # f extent per super-batch
    s_act = n_act * D  # flat f extent handled by the square pipeline
    assert s_act % SUPER == 0 and (NS - n_act) % 128 == 0
    n_super = s_act // SUPER

    n_dve = NS - n_act
    NU = n_dve // 128  # 128-n chunks on the DVE side

    with tile.TileContext(nc) as tc:
        with (
            tc.tile_pool(name="wpool", bufs=1) as wpool,
            tc.tile_pool(name="vpool", bufs=2) as vpool,
            tc.tile_pool(name="qpool", bufs=3) as qpool,
            tc.tile_pool(name="stage", bufs=2) as stage_pool,
            tc.tile_pool(name="dvep", bufs=1) as dvepool,
            tc.tile_pool(name="dtmp", bufs=3) as dtmp_pool,
            tc.tile_pool(name="sdp", bufs=2, space="PSUM") as sd_pool,
            tc.tile_pool(name="outp", bufs=2, space="PSUM") as outp_pool,
        ):
            if n_act:
                gt4 = wpool.tile([128, R], BF16, name="gt4")
                for i in range(4):
                    nc.gpsimd.dma_start(gt4[32 * i : 32 * i + 18, :], g_d[:])
                wt = wpool.tile([R, 9], BF16, name="wt")
                nc.gpsimd.dma_start(wt[:], w_d[:])

            # ---- DVE-side setup: tiles, loads, and the deferred op list ----
            dve_ops = []
            if NU:
                x1t = dvepool.tile([128, NU, M, D], BF16, name="x1t")
                x2t = dvepool.tile([128, NU, M, D], BF16, name="x2t")
                accs = [
                    dvepool.tile([128, NU, D], F32, name=f"dacc{m}", tag=f"dacc{m}")
                    for m in range(M)
                ]

                def _dve_load(u):
                    n0 = n_act + u * 128
                    nc.gpsimd.dma_start(
                        x1t[:, u, :, :],
                        x1_d[:, n0 : n0 + 128, :].rearrange("m n d -> n m d"),
                    )
                    nc.gpsimd.dma_start(
                        x2t[:, u, :, :],
                        x2_d[:, n0 : n0 + 128, :].rearrange("m n d -> n m d"),
                    )

                written = [False] * M

                def _stt(out_ap, in0_ap, w, in1_ap, op1):
                    return lambda: nc.vector.scalar_tensor_tensor(
                        out_ap, in0_ap, w, in1_ap, MULT, op1
                    )

                for a, b, uses in plan:
                    x1a = x1t[:, :, a, :]
                    x2b = x2t[:, :, b, :]
                    if len(uses) == 1 and not written[uses[0][0]]:
                        m, w = uses[0]
                        dve_ops.append(_stt(accs[m][:], x1a, float(w), x2b, MULT))
                        written[m] = True
                    else:
                        # product into tmp, then one fused op per use
                        def _mk(x1a=x1a, x2b=x2b, uses=tuple(uses)):
                            ops = []
                            state = {}
                            def prod():
                                state["tmp"] = dtmp_pool.tile(
                                    [128, NU, D], BF16, name="dtmp"
                                )
                                nc.vector.tensor_mul(state["tmp"][:], x1a, x2b)
                            ops.append(prod)
                            for m, w in uses:
                                if not written[m]:
                                    def fw(m=m, w=w):
                                        nc.vector.tensor_scalar_mul(
                                            accs[m][:], state["tmp"][:], float(w)
                                        )
                                    ops.append(fw)
                                    written[m] = True
                                else:
                                    def ac(m=m, w=w):
                                        nc.vector.scalar_tensor_tensor(
                                            accs[m][:], state["tmp"][:], float(w),
                                            accs[m][:], MULT, ADD,
                                        )
                                    ops.append(ac)
                            return ops
                        dve_ops.extend(_mk())

                for m in range(M):
                    if not written[m]:
                        mm = m
                        dve_ops.insert(0, lambda mm=mm: nc.vector.memset(accs[mm][:], 0.0))

            def _emit_dve(k0, k1):
                for op in dve_ops[k0:k1]:
                    op()

            # Chunk (i, b) of super-batch sb covers f-range
            #   [f0 + (i*NB + b)*F1, +F1)  with f0 = sb*SUPER —
            # row-group i owns a CONTIGUOUS [9, NB*F1] HBM range (16KB/row).
            ndve_done = 0
            for sb in range(n_super):
                f0 = sb * SUPER
                v4 = vpool.tile([128, NB * F1], BF16, name="v4")
                for i in range(4):
                    fi = f0 + i * NB * F1
                    nc.gpsimd.dma_start(
                        v4[32 * i : 32 * i + 9, :], x1_f[:, fi : fi + NB * F1]
                    )
                    nc.gpsimd.dma_start(
                        v4[32 * i + 9 : 32 * i + 18, :], x2_f[:, fi : fi + NB * F1]
                    )
                if sb == 0 and NU:
                    # DVE-side loads AFTER the first square-batch loads so the
                    # sq pipeline's first tiles aren't stuck behind 4.8MB of
                    # DVE-side DMA in the SWDGE queues.
                    for u in range(NU):
                        _dve_load(u)

                stage = stage_pool.tile([128, NB * F1], F32, name="stage")

                # Process in half-batches of 2 chunks (2 PSUM banks) so the
                # SD pool double-buffers within the 8-bank budget:
                # groups (0,1) then (2,3) for each b.
                for b in range(NB):
                    outp = outp_pool.tile([128, F1], F32, name="outp", tag="outp")
                    for h in range(2):
                        sd = sd_pool.tile([R, 2, F1], F32, name="sd")
                        for j in range(2):
                            i = 2 * h + j
                            nc.tensor.matmul(
                                sd[:, j, :],
                                gt4[32 * i : 32 * i + 18, :],
                                v4[32 * i : 32 * i + 18, b * F1 : (b + 1) * F1],
                                start=True,
                                stop=True,
                                tile_position=(32 * i, 0),
                            )
                        qs = qpool.tile([R, 2, F1], BF16, name="qs")
                        nc.scalar.activation(
                            qs[:].rearrange("p a t -> p (a t)"),
                            sd[:].rearrange("p a t -> p (a t)"),
                            SQUARE,
                        )
                        for j in range(2):
                            i = 2 * h + j
                            nc.tensor.matmul(
                                outp[32 * i : 32 * i + 9, :],
                                wt[:],
                                qs[:, j, :],
                                start=True,
                                stop=True,
                                tile_position=(0, 32 * i),
                            )
                        if h == 1:
                            nc.scalar.copy(
                                stage[:, b * F1 : (b + 1) * F1], outp[:]
                            )

                for i in range(4):
                    fi = f0 + i * NB * F1
                    nc.sync.dma_start(
                        out_f[:, fi : fi + NB * F1], stage[32 * i : 32 * i + 9, :]
                    )

                k1 = len(dve_ops) * (sb + 1) // max(n_super, 1)
                _emit_dve(ndve_done, k1)
                ndve_done = k1

            if n_super == 0 and NU:
                for u in range(NU):
                    _dve_load(u)
            _emit_dve(ndve_done, len(dve_ops))
            if NU:
                for u in range(NU):
                    n0 = n_act + u * 128
                    for m in range(M):
                        nc.sync.dma_start(
                            out_d[m, n0 : n0 + 128, :], accs[m][:, u, :]
                        )

    nc.compile()
    return nc


_CACHE = {}


def _get_nc(key, builder, *args):
    if key not in _CACHE:
        _CACHE[key] = builder(*args)
    return _CACHE[key]


def prepare(X1, X2, m1_aligned, m2_aligned, mu, multipliers):
    """Build (or fetch cached) the Bass program and per-core input maps."""
    X1 = np.ascontiguousarray(X1, dtype=np.float32)
    X2 = np.ascontiguousarray(X2, dtype=np.float32)
    m1 = [int(v) for v in np.asarray(m1_aligned)]
    m2 = [int(v) for v in np.asarray(m2_aligned)]
    mus = [int(v) for v in np.asarray(mu)]
    mult = [float(v) for v in np.asarray(multipliers, dtype=np.float32)]

    impl = os.environ.get("BASS_KERNEL_IMPL", "sq")
    n_act = int(os.environ.get("BASS_NA", "256"))
    key = (impl, n_act, tuple(m1), tuple(m2), tuple(mus), tuple(mult))
    if impl == "dve1":
        nc = _get_nc(key, _build_dve_kernel, m1, m2, mus, mult)
        extra = {}
    else:
        nc = _get_nc(key, _build_sq_kernel, m1, m2, mus, mult, n_act)
        G_np, W_np, _ = _sq_matrices(m1, m2, mus, mult)
        extra = {"G": G_np, "W": W_np}

    in_maps = []
    for c in range(NCORES):
        sl = slice(c * NS, (c + 1) * NS)
        in_maps.append(
            {
                "X1": np.ascontiguousarray(X1[:, sl, :]),
                "X2": np.ascontiguousarray(X2[:, sl, :]),
                **extra,
            }
        )
    return nc, in_maps


def kernel(X1, X2, m1_aligned, m2_aligned, mu, multipliers):
    nc, in_maps = prepare(X1, X2, m1_aligned, m2_aligned, mu, multipliers)
    res = run_bass_kernel_spmd(nc, in_maps, core_ids=list(range(NCORES)))
    out = np.concatenate([res.results[c]["OUT"] for c in range(NCORES)], axis=1)
    return out
